# revision 6
# baseline (speedup 1.0000x reference)
"""ComplexPolarAttention Trainium2 kernel.

score_ij = sum_d mag_i,d mag_j,d cos(phase_i,d - phase_j,d)
         = a_i . a_j + b_i . b_j          with a = mag*cos(phase), b = mag*sin(phase)
out_mag   = softmax(score, axis=1) @ mag
out_phase = softmax(score, axis=1) @ phase

Strategy (8 NeuronCores, SPMD, no collectives):
  - Rows (queries) sharded; keys replicated. Per-core inputs are ROTATED
    along the key axis so core c's queries are always key columns 0..q of its
    own panel (softmax over keys is permutation invariant), so the query
    operand is a prefix of the key panel.
  - The packed ab^T = [a|b]^T bf16 panel is PRESCALED by sqrt(A), A=2^7/ln2,
    so the score matmuls produce s' = A*s in PSUM. This lets exp run on TWO
    engines: ACT computes exp(s'*(1/A)) (free scale), and DVE computes a
    Schraudolph exp: round(s' + B) written as int16 IS the bf16 bit pattern
    of ~exp(s) (B = 127*2^7 + C, C tuned so mean(es_schr/exp) ~ 1; rel rms
    ~1.8% which softmax-averages to ~5e-3 output error). The es tile is
    bitcast back to bf16 for the value matmuls. DVE handles ~10 key blocks,
    ACT the rest; both run under the TensorE roof (~57us), which is the
    bf16 matmul floor (4 x [128,512] @ 216ns per key block).
  - Value matmuls: packed bf16 stationary [mag|phase] per key block, es as
    the 2x512 moving operand, accumulated over all 64 key blocks into two
    [128,512] fp32 PSUM banks. Value matmuls LAG 3 blocks behind the score
    matmuls so exp latency never stalls TensorE.
  - Softmax denominator: GpSimd owns a bf16 running sum W_G over ~15 blocks;
    DVE runs double-buffered bf16 chains of ~11 over the rest, flushed into
    an fp32 master (GpSimd mid-stream, DVE for the last). psD = ones^T @
    (master + W_G + last NDIRECT blocks' es directly).
  - Outputs are DMA'd STRAIGHT FROM PSUM (accA/accB and psD0/psD1) on four
    different engine queues so the tail is ~2us. The final divide is on host.
  - Input DMA: the query panel (cols 0:1024) arrives as two 512-wide chunks
    on the sync queue so the first score matmul starts ~7.5us; the remaining
    key panel in 3 big chunks behind it; the value matrix in 4 chunk-major
    pieces on the vector/gpsimd queues.
"""

import numpy as np
from contextlib import ExitStack

import concourse.bass as bass
import concourse.tile as tile
from concourse import bacc, mybir
from concourse.bass_utils import run_bass_kernel_spmd

F32 = mybir.dt.float32
F32R = mybir.dt.float32r
BF16 = mybir.dt.bfloat16
I16 = mybir.dt.int16

A_EXP = 2.0 ** 7 / float(np.log(2.0))     # 184.665
SQRT_A = float(np.sqrt(A_EXP))
B_SCHR = 127.0 * 128.0 - 7.36             # 16256 + C, C tuned for mean ratio 1

LAG = 3             # value matmuls trail score matmuls by LAG blocks
SPLIT_HEAD = 2      # first blocks exp in 512-col halves (earlier ACT start)
NDIRECT = 3         # trailing blocks folded straight into psD matmuls
CHAIN = 11          # DVE bf16 partial-sum chain length


def build_program(n=8192, d=64, n_cores=8, enable_asserts=False):
    """Build the SPMD Bass program. Every core runs identical IR; per-core
    behavior comes only from per-core (rotated) input data."""
    assert d == 64
    q = n // n_cores            # queries per core
    kblocks = n // 128          # key blocks of 128
    qblk = q // 2               # half per matmul (psum bank = 512 fp32)
    assert qblk <= 512 and n % 128 == 0 and q <= 1024

    last = kblocks - 1
    dve_exp = {kb for kb in range(SPLIT_HEAD, last)
               if kb % 6 == 3}                       # ~10 Schraudolph blocks
    gp_chain = {kb for kb in range(kblocks - NDIRECT) if kb % 4 == 1}
    # DVE chain bookkeeping: chained blocks not owned by GpSimd
    dve_blocks = [kb for kb in range(kblocks - NDIRECT) if kb not in gp_chain]
    dve_pos = {kb: (i // CHAIN, i % CHAIN) for i, kb in enumerate(dve_blocks)}
    nchain = (len(dve_blocks) + CHAIN - 1) // CHAIN
    chain_last = {}
    for i, kb in enumerate(dve_blocks):
        if i % CHAIN == CHAIN - 1 or i == len(dve_blocks) - 1:
            chain_last[kb] = dve_pos[kb][0]

    nc = bacc.Bacc(
        "TRN2",
        target_bir_lowering=False,
        debug=False,
        enable_asserts=enable_asserts,
        num_devices=n_cores,
    )

    # ---- DRAM I/O (all per-core arrays rotated so queries = keys[0:q]) ----
    abq_in = [nc.dram_tensor(f"abq{i}", [128, qblk], BF16,
                             kind="ExternalInput").ap() for i in range(2)]
    rest = n - q
    rest_w = [2048, 2048, rest - 4096] if rest > 4096 else [rest]
    abt_in = [nc.dram_tensor(f"abt{i}", [128, w], BF16,
                             kind="ExternalInput").ap()
              for i, w in enumerate(rest_w)]
    vchunk = 16
    nvch = kblocks // vchunk
    vt = nc.dram_tensor("vt", [nvch, 128, vchunk * 128], BF16,
                        kind="ExternalInput").ap()
    ones_in = nc.dram_tensor("onesv", [128, 1], F32R,
                             kind="ExternalInput").ap()
    onesb_in = nc.dram_tensor("onesb", [128, 1], BF16,
                              kind="ExternalInput").ap()

    onumA = nc.dram_tensor("onumA", [128, qblk], F32, kind="ExternalOutput").ap()
    onumB = nc.dram_tensor("onumB", [128, qblk], F32, kind="ExternalOutput").ap()
    odenA = nc.dram_tensor("odenA", [1, qblk], F32, kind="ExternalOutput").ap()
    odenB = nc.dram_tensor("odenB", [1, qblk], F32, kind="ExternalOutput").ap()

    with tile.TileContext(nc) as tc, ExitStack() as ctx:
        persist = ctx.enter_context(tc.tile_pool(name="persist", bufs=1))
        epool = ctx.enter_context(tc.tile_pool(name="exps", bufs=7))
        spool = ctx.enter_context(tc.tile_pool(name="scores", bufs=3, space="PSUM"))
        apool = ctx.enter_context(tc.tile_pool(name="accum", bufs=1, space="PSUM"))

        abq = persist.tile([128, q], BF16)       # query panel = keys[0:q]
        abt_r = persist.tile([128, rest], BF16)  # keys[q:n]
        vt_t = persist.tile([128, kblocks, 128], BF16)
        ones = persist.tile([128, 1], F32R)
        ones_bf = persist.tile([128, 1], BF16)
        wd = [persist.tile([128, q], BF16, name=f"wd{i}") for i in range(2)]
        wg = persist.tile([128, q], BF16)        # GpSimd-owned running sum
        master = persist.tile([128, q], F32R)    # fp32 master sum

        # zero-init running sums while engines are idle in the preamble
        # (f32r memset fails the ISA value-type check; memset the raw bits)
        nc.vector.memset(master.bitcast(mybir.dt.int32)[:, :], 0)
        nc.vector.memset(wg[:, :], 0.0)

        # ---- input DMA: abq first (gates the first matmul), on sync ----
        for i in range(2):
            nc.sync.dma_start(out=abq[:, i * qblk:(i + 1) * qblk], in_=abq_in[i])
        off = 0
        for i, w in enumerate(rest_w):
            nc.sync.dma_start(out=abt_r[:, off:off + w], in_=abt_in[i])
            off += w
        # value matrix: chunk 0 on the scalar queue (lands before the first
        # value matmul at ~LAG blocks in), the rest behind it on gpsimd
        nc.scalar.dma_start(out=vt_t[:, 0:vchunk, :], in_=vt[0, :, :])
        for vi in range(1, nvch):
            nc.gpsimd.dma_start(out=vt_t[:, vi * vchunk:(vi + 1) * vchunk, :],
                                in_=vt[vi, :, :])
        nc.gpsimd.dma_start(out=ones[:, :], in_=ones_in)
        nc.gpsimd.dma_start(out=ones_bf[:, :], in_=onesb_in)

        accA = apool.tile([128, qblk], F32, name="accA", tag="accA")
        accB = apool.tile([128, qblk], F32, name="accB", tag="accB")

        def keyblk(kb):
            c0 = kb * 128
            if c0 + 128 <= q:
                return abq[:, c0:c0 + 128]
            return abt_r[:, c0 - q:c0 - q + 128]

        def chain_op(kb, eh):
            """Denominator accumulation for block kb's es tile."""
            if kb >= kblocks - NDIRECT:
                return
            t1, o1, t2, o2 = eh
            if kb in gp_chain:
                w, eng, first = wg, nc.gpsimd, False   # memset-initialized
            else:
                c, ci = dve_pos[kb]
                w, eng, first = wd[c % 2], nc.vector, (ci == 0)
            if t1 is t2:
                pieces = [(w[:, :], t1[:, o1:o1 + q])]
            else:
                pieces = [(w[:, 0:qblk], t1[:, o1:o1 + qblk]),
                          (w[:, qblk:q], t2[:, o2:o2 + qblk])]
            for dst, src in pieces:
                if first:
                    eng.tensor_copy(dst, src)
                else:
                    eng.tensor_add(dst, dst, src)
            c = chain_last.get(kb)
            if c is not None:   # flush this chain into the fp32 master
                eng2 = nc.vector if c == nchain - 1 else nc.gpsimd
                eng2.tensor_add(master[:, :], master[:, :], wd[c % 2][:, :])

        def value_mms(kb, eh):
            first, lastb = (kb == 0), (kb == kblocks - 1)
            for j in range(2):
                acc = accA if j == 0 else accB
                t, o = eh[2 * j], eh[2 * j + 1]
                nc.tensor.matmul(
                    out=acc[:, :], lhsT=vt_t[:, kb, :],
                    rhs=t[:, o:o + qblk],
                    start=first, stop=lastb)
            chain_op(kb, eh)

        es_hist = []
        for kb in range(kblocks):
            if kb >= LAG:
                value_mms(kb - LAG, es_hist[kb - LAG])
            ss = spool.tile([128, q], F32)
            split = kb < SPLIT_HEAD or kb == last
            halves = []
            for j in range(2):
                qsl = slice(j * qblk, (j + 1) * qblk)
                nc.tensor.matmul(
                    out=ss[:, qsl],
                    lhsT=keyblk(kb),
                    rhs=abq[:, qsl],
                    start=True, stop=True,
                )
                if split:
                    e = epool.tile([128, qblk], BF16)
                    nc.scalar.activation(
                        e[:, :], ss[:, qsl],
                        mybir.ActivationFunctionType.Exp, scale=1.0 / A_EXP)
                    halves.append(e)
            if split:
                es_hist.append((halves[0], 0, halves[1], 0))
            elif kb in dve_exp:
                es = epool.tile([128, q], I16)
                nc.vector.tensor_scalar(
                    out=es[:, :], in0=ss[:, :], scalar1=B_SCHR, scalar2=None,
                    op0=mybir.AluOpType.add)
                esb = es.bitcast(BF16)
                es_hist.append((esb, 0, esb, qblk))
            else:
                es = epool.tile([128, q], BF16)
                nc.scalar.activation(
                    es[:, :], ss[:, :], mybir.ActivationFunctionType.Exp,
                    scale=1.0 / A_EXP)
                es_hist.append((es, 0, es, qblk))

        # ---- tail: remaining value matmuls + denominator psD ----
        value_mms(kblocks - 3, es_hist[-3])
        psD = []
        for j in range(2):
            qsl = slice(j * qblk, (j + 1) * qblk)
            pd = spool.tile([1, qblk], F32, name=f"psD{j}", tag="ss")
            nc.tensor.matmul(out=pd[:, :], lhsT=ones[:, :],
                             rhs=master[:, qsl], start=True, stop=False)
            nc.tensor.matmul(out=pd[:, :], lhsT=ones_bf[:, :],
                             rhs=wg[:, qsl], start=False, stop=False)
            psD.append(pd)
        for kb in range(kblocks - NDIRECT, kblocks - 2):
            for j in range(2):
                t, o = es_hist[kb][2 * j], es_hist[kb][2 * j + 1]
                nc.tensor.matmul(out=psD[j][:, :], lhsT=ones_bf[:, :],
                                 rhs=t[:, o:o + qblk],
                                 start=False, stop=False)
        value_mms(kblocks - 2, es_hist[-2])
        for j in range(2):
            t, o = es_hist[-2][2 * j], es_hist[-2][2 * j + 1]
            nc.tensor.matmul(out=psD[j][:, :], lhsT=ones_bf[:, :],
                             rhs=t[:, o:o + qblk], start=False, stop=False)
        value_mms(kblocks - 1, es_hist[-1])
        for j in range(2):
            t, o = es_hist[-1][2 * j], es_hist[-1][2 * j + 1]
            nc.tensor.matmul(out=psD[j][:, :], lhsT=ones_bf[:, :],
                             rhs=t[:, o:o + qblk], start=False, stop=True)

        # ---- outputs: PSUM -> SBUF (DVE half A, ACT half B) -> DMA; the
        # small den copies go first so the oden DMAs launch early
        opool = ctx.enter_context(tc.tile_pool(name="outs", bufs=1))
        oDa = opool.tile([1, qblk], F32, name="oDa")
        oDb = opool.tile([1, qblk], F32, name="oDb")
        nc.vector.tensor_copy(oDa[:, :], psD[0][:, :])
        nc.scalar.activation(oDb[:, :], psD[1][:, :],
                             mybir.ActivationFunctionType.Copy)
        nc.gpsimd.dma_start(out=odenA, in_=oDa[:, :])
        nc.gpsimd.dma_start(out=odenB, in_=oDb[:, :])
        oNa = opool.tile([128, qblk], F32, name="oNa")
        oNb = opool.tile([128, qblk], F32, name="oNb")
        nc.vector.tensor_copy(oNa[:, :], accA[:, :])
        nc.scalar.activation(oNb[:, :], accB[:, :],
                             mybir.ActivationFunctionType.Copy)
        nc.sync.dma_start(out=onumA, in_=oNa[:, :])
        nc.sync.dma_start(out=onumB, in_=oNb[:, :])

    nc.compile()
    return nc


def make_inputs(mag, phase, n_cores=8):
    """Host-side sharding/layout prep -> per-core (key-rotated) input maps."""
    import ml_dtypes
    bf16 = ml_dtypes.bfloat16
    n, d = mag.shape
    q = n // n_cores
    qblk = q // 2
    kblocks = n // 128
    mag = np.ascontiguousarray(mag, dtype=np.float32)
    phase = np.ascontiguousarray(phase, dtype=np.float32)

    a = mag * np.cos(phase)
    b = mag * np.sin(phase)
    abt_g = (np.concatenate([a.T, b.T], axis=0) * SQRT_A).astype(bf16)
    v_nat = np.concatenate([mag, phase], axis=1).astype(bf16)   # [n, 128]

    rest = n - q
    rest_w = [2048, 2048, rest - 4096] if rest > 4096 else [rest]
    vchunk = 16
    nvch = kblocks // vchunk

    def tile_nat(x):  # [n, m] -> [nvch, 128, vchunk*m] chunk-major
        m = x.shape[1]
        y = x.reshape(nvch, vchunk, 128, m).transpose(0, 2, 1, 3)
        return np.ascontiguousarray(y.reshape(nvch, 128, vchunk * m))

    in_maps = []
    for c in range(n_cores):
        r = c * q
        abt_c = np.roll(abt_g, -r, axis=1)
        m = {"vt": tile_nat(np.roll(v_nat, -r, axis=0)),
             "onesv": np.ones((128, 1), np.float32),
             "onesb": np.ones((128, 1), bf16)}
        for i in range(2):
            m[f"abq{i}"] = np.ascontiguousarray(
                abt_c[:, i * qblk:(i + 1) * qblk])
        off = q
        for i, w in enumerate(rest_w):
            m[f"abt{i}"] = np.ascontiguousarray(abt_c[:, off:off + w])
            off += w
        in_maps.append(m)
    return in_maps


def gather_outputs(results, n, d, n_cores=8):
    """Per-core transposed unnormalized sums + denominators -> full outputs."""
    new_mag = np.empty((n, d), np.float32)
    new_phase = np.empty((n, d), np.float32)
    q = n // n_cores
    qblk = q // 2
    for c in range(n_cores):
        r = results[c]
        for j, (nm, dn) in enumerate((("onumA", "odenA"), ("onumB", "odenB"))):
            onum = np.asarray(r[nm]).astype(np.float32)   # [128, qblk]
            den = np.asarray(r[dn]).astype(np.float32)    # [1, qblk]
            qsl = slice(c * q + j * qblk, c * q + (j + 1) * qblk)
            new_mag[qsl] = (onum[:64, :] / den).T
            new_phase[qsl] = (onum[64:128, :] / den).T
    return new_mag, new_phase


_PROGRAM_CACHE = {}


def _get_program(n, d, n_cores):
    key = (n, d, n_cores)
    if key not in _PROGRAM_CACHE:
        _PROGRAM_CACHE[key] = build_program(n=n, d=d, n_cores=n_cores)
    return _PROGRAM_CACHE[key]


def kernel(mag, phase):
    mag = np.asarray(mag, dtype=np.float32)
    phase = np.asarray(phase, dtype=np.float32)
    n, d = mag.shape
    n_cores = 8
    nc = _get_program(n, d, n_cores)
    in_maps = make_inputs(mag, phase, n_cores=n_cores)
    res = run_bass_kernel_spmd(nc, in_maps, list(range(n_cores)))
    return gather_outputs(res.results, n, d, n_cores=n_cores)


# revision 13
# speedup vs baseline: 1.2643x; 1.2643x over previous
"""ComplexPolarAttention Trainium2 kernel.

score_ij = sum_d mag_i,d mag_j,d cos(phase_i,d - phase_j,d)
         = a_i . a_j + b_i . b_j          with a = mag*cos(phase), b = mag*sin(phase)
out_mag   = softmax(score, axis=1) @ mag
out_phase = softmax(score, axis=1) @ phase

Strategy (8 NeuronCores, SPMD, no collectives):
  - Rows (queries) sharded; keys replicated. Per-core inputs are ROTATED
    along the key axis so core c's queries are always key columns 0..q of its
    own panel (softmax over keys is permutation invariant), so the query
    operand is a prefix of the key panel.
  - The packed ab^T = [a|b]^T bf16 panel is PRESCALED by sqrt(A), A=2^7/ln2,
    so the score matmuls produce s' = A*s in PSUM. This lets exp run on TWO
    engines: ACT computes exp(s'*(1/A)) (free scale), and DVE computes a
    Schraudolph exp: round(s' + B) written as int16 IS the bf16 bit pattern
    of ~exp(s) (B = 127*2^7 + C, C tuned so mean(es_schr/exp) ~ 1; rel rms
    ~1.8% which softmax-averages to ~5e-3 output error). The es tile is
    bitcast back to bf16 for the value matmuls. DVE handles ~10 key blocks,
    ACT the rest; both run under the TensorE roof (~57us), which is the
    bf16 matmul floor (4 x [128,512] @ 216ns per key block).
  - Value matmuls: packed bf16 stationary [mag|phase] per key block, es as
    the 2x512 moving operand, accumulated over all 64 key blocks into two
    [128,512] fp32 PSUM banks. Value matmuls LAG 3 blocks behind the score
    matmuls so exp latency never stalls TensorE.
  - Softmax denominator: GpSimd owns a bf16 running sum W_G over ~15 blocks;
    DVE runs double-buffered bf16 chains of ~11 over the rest, flushed into
    an fp32 master (GpSimd mid-stream, DVE for the last). psD = ones^T @
    (master + W_G + last NDIRECT blocks' es directly).
  - Outputs are DMA'd STRAIGHT FROM PSUM (accA/accB and psD0/psD1) on four
    different engine queues so the tail is ~2us. The final divide is on host.
  - Input DMA: the query panel (cols 0:1024) arrives as two 512-wide chunks
    on the sync queue so the first score matmul starts ~7.5us; the remaining
    key panel in 3 big chunks behind it; the value matrix in 4 chunk-major
    pieces on the vector/gpsimd queues.
"""

import numpy as np
from contextlib import ExitStack

import concourse.bass as bass
import concourse.tile as tile
from concourse import bacc, mybir
from concourse.bass_utils import run_bass_kernel_spmd

F32 = mybir.dt.float32
F32R = mybir.dt.float32r
BF16 = mybir.dt.bfloat16
I16 = mybir.dt.int16

A_EXP = 2.0 ** 7 / float(np.log(2.0))     # 184.665
SQRT_A = float(np.sqrt(A_EXP))
B_SCHR = 127.0 * 128.0 - 7.36             # 16256 + C, C tuned for mean ratio 1

LAG = 3             # value matmuls trail score matmuls by LAG blocks
SPLIT_HEAD = 2      # first blocks exp in 512-col halves (earlier ACT start)
NDIRECT = 3         # trailing blocks folded straight into psD matmuls
CHAIN = 16          # DVE bf16 partial-sum chain length


def build_program(n=8192, d=64, n_cores=8, enable_asserts=False):
    """Build the SPMD Bass program. Every core runs identical IR; per-core
    behavior comes only from per-core (rotated) input data."""
    assert d == 64
    q = n // n_cores            # queries per core
    kblocks = n // 128          # key blocks of 128
    qblk = q // 2               # half per matmul (psum bank = 512 fp32)
    assert qblk <= 512 and n % 128 == 0 and q <= 1024

    last = kblocks - 1
    # GpSimd tensor ops cost ~0.5 DVE-us per GpSimd-us in SBUF contention
    # (measured), so GpSimd gets NO tensor work; exp/den balance is ACT+DVE.
    dve_exp = {21, 43} & set(range(SPLIT_HEAD, last))
    dve_blocks = [kb for kb in range(kblocks - NDIRECT)]
    dve_pos = {kb: (i // CHAIN, i % CHAIN) for i, kb in enumerate(dve_blocks)}
    nchain = (len(dve_blocks) + CHAIN - 1) // CHAIN
    chain_last = {}
    for i, kb in enumerate(dve_blocks):
        if i % CHAIN == CHAIN - 1 or i == len(dve_blocks) - 1:
            chain_last[kb] = dve_pos[kb][0]

    nc = bacc.Bacc(
        "TRN2",
        target_bir_lowering=False,
        debug=False,
        enable_asserts=enable_asserts,
        num_devices=n_cores,
    )

    # ---- DRAM I/O (all per-core arrays rotated so queries = keys[0:q]) ----
    abq_in = [nc.dram_tensor(f"abq{i}", [128, qblk], BF16,
                             kind="ExternalInput").ap() for i in range(2)]
    rest = n - q
    rest_w = ([1024, 1024, 2048, rest - 4096] if rest > 4096 else [rest])
    abt_in = [nc.dram_tensor(f"abt{i}", [128, w], BF16,
                             kind="ExternalInput").ap()
              for i, w in enumerate(rest_w)]
    # value chunks: small early (land before first value matmuls), big later
    vch_w = [4, 4, 8, 16, 16, 16]
    assert sum(vch_w) == kblocks
    vt_in = [nc.dram_tensor(f"vt{i}", [128, w * 128], BF16,
                            kind="ExternalInput").ap()
             for i, w in enumerate(vch_w)]
    ones_in = nc.dram_tensor("onesv", [128, 1], F32R,
                             kind="ExternalInput").ap()
    onesb_in = nc.dram_tensor("onesb", [128, 1], BF16,
                              kind="ExternalInput").ap()

    onumA = nc.dram_tensor("onumA", [128, qblk], F32, kind="ExternalOutput").ap()
    onumB = nc.dram_tensor("onumB", [128, qblk], F32, kind="ExternalOutput").ap()
    odenA = nc.dram_tensor("odenA", [1, qblk], F32, kind="ExternalOutput").ap()
    odenB = nc.dram_tensor("odenB", [1, qblk], F32, kind="ExternalOutput").ap()

    with tile.TileContext(nc) as tc, ExitStack() as ctx:
        persist = ctx.enter_context(tc.tile_pool(name="persist", bufs=1))
        epool = ctx.enter_context(tc.tile_pool(name="exps", bufs=7))
        spool = ctx.enter_context(tc.tile_pool(name="scores", bufs=3, space="PSUM"))
        apool = ctx.enter_context(tc.tile_pool(name="accum", bufs=1, space="PSUM"))

        abq = persist.tile([128, q], BF16)       # query panel = keys[0:q]
        abt_r = persist.tile([128, rest], BF16)  # keys[q:n]
        vt_t = persist.tile([128, kblocks, 128], BF16)
        ones = persist.tile([128, 1], F32R)
        ones_bf = persist.tile([128, 1], BF16)
        wd = [persist.tile([128, q], BF16, name=f"wd{i}") for i in range(2)]
        master = persist.tile([128, q], F32R)    # fp32 master sum

        # zero-init the master while engines are idle in the preamble
        # (f32r memset fails the ISA value-type check; memset the raw bits)
        nc.vector.memset(master.bitcast(mybir.dt.int32)[:, :], 0)

        # ---- input DMA: abq first (gates the first matmul), on sync ----
        for i in range(2):
            nc.sync.dma_start(out=abq[:, i * qblk:(i + 1) * qblk], in_=abq_in[i])
        off = 0
        for i, w in enumerate(rest_w):
            nc.sync.dma_start(out=abt_r[:, off:off + w], in_=abt_in[i])
            off += w
        # value matrix on gpsimd (idle: no tensor work assigned to it)
        vb = 0
        for i, w in enumerate(vch_w):
            nc.gpsimd.dma_start(out=vt_t[:, vb:vb + w, :], in_=vt_in[i])
            vb += w
        nc.gpsimd.dma_start(out=ones[:, :], in_=ones_in)
        nc.gpsimd.dma_start(out=ones_bf[:, :], in_=onesb_in)

        accA = apool.tile([128, qblk], F32, name="accA", tag="accA")
        accB = apool.tile([128, qblk], F32, name="accB", tag="accB")

        def keyblk(kb):
            c0 = kb * 128
            if c0 + 128 <= q:
                return abq[:, c0:c0 + 128]
            return abt_r[:, c0 - q:c0 - q + 128]

        def chain_op(kb, eh):
            """Denominator accumulation for block kb's es tile (all DVE)."""
            if kb >= kblocks - NDIRECT:
                return
            t1, o1, t2, o2 = eh
            c, ci = dve_pos[kb]
            w, first = wd[c % 2], (ci == 0)
            if t1 is t2:
                pieces = [(w[:, :], t1[:, o1:o1 + q])]
            else:
                pieces = [(w[:, 0:qblk], t1[:, o1:o1 + qblk]),
                          (w[:, qblk:q], t2[:, o2:o2 + qblk])]
            for dst, src in pieces:
                if first:
                    nc.vector.tensor_copy(dst, src)
                else:
                    nc.vector.tensor_add(dst, dst, src)
            c = chain_last.get(kb)
            if c is not None:   # flush this chain into the fp32 master
                nc.vector.tensor_add(master[:, :], master[:, :],
                                     wd[c % 2][:, :])

        def value_mms(kb, eh):
            first, lastb = (kb == 0), (kb == kblocks - 1)
            for j in range(2):
                acc = accA if j == 0 else accB
                t, o = eh[2 * j], eh[2 * j + 1]
                nc.tensor.matmul(
                    out=acc[:, :], lhsT=vt_t[:, kb, :],
                    rhs=t[:, o:o + qblk],
                    start=first, stop=lastb)
            chain_op(kb, eh)

        es_hist = []
        for kb in range(kblocks):
            if kb >= LAG:
                value_mms(kb - LAG, es_hist[kb - LAG])
            ss = spool.tile([128, q], F32)
            split = kb < SPLIT_HEAD or kb == last
            halves = []
            for j in range(2):
                qsl = slice(j * qblk, (j + 1) * qblk)
                nc.tensor.matmul(
                    out=ss[:, qsl],
                    lhsT=keyblk(kb),
                    rhs=abq[:, qsl],
                    start=True, stop=True,
                )
                if split:
                    e = epool.tile([128, qblk], BF16)
                    nc.scalar.activation(
                        e[:, :], ss[:, qsl],
                        mybir.ActivationFunctionType.Exp, scale=1.0 / A_EXP)
                    halves.append(e)
            if split:
                es_hist.append((halves[0], 0, halves[1], 0))
            elif kb in dve_exp:
                es = epool.tile([128, q], I16)
                nc.vector.tensor_scalar(
                    out=es[:, :], in0=ss[:, :], scalar1=B_SCHR, scalar2=None,
                    op0=mybir.AluOpType.add)
                esb = es.bitcast(BF16)
                es_hist.append((esb, 0, esb, qblk))
            else:
                es = epool.tile([128, q], BF16)
                nc.scalar.activation(
                    es[:, :], ss[:, :], mybir.ActivationFunctionType.Exp,
                    scale=1.0 / A_EXP)
                es_hist.append((es, 0, es, qblk))

        # ---- tail: remaining value matmuls + denominator psD ----
        value_mms(kblocks - 3, es_hist[-3])
        psD = []
        for j in range(2):
            qsl = slice(j * qblk, (j + 1) * qblk)
            pd = spool.tile([1, qblk], F32, name=f"psD{j}", tag="ss")
            nc.tensor.matmul(out=pd[:, :], lhsT=ones[:, :],
                             rhs=master[:, qsl], start=True, stop=False)
            psD.append(pd)
        for kb in range(kblocks - NDIRECT, kblocks - 2):
            for j in range(2):
                t, o = es_hist[kb][2 * j], es_hist[kb][2 * j + 1]
                nc.tensor.matmul(out=psD[j][:, :], lhsT=ones_bf[:, :],
                                 rhs=t[:, o:o + qblk],
                                 start=False, stop=False)
        value_mms(kblocks - 2, es_hist[-2])
        for j in range(2):
            t, o = es_hist[-2][2 * j], es_hist[-2][2 * j + 1]
            nc.tensor.matmul(out=psD[j][:, :], lhsT=ones_bf[:, :],
                             rhs=t[:, o:o + qblk], start=False, stop=False)
        value_mms(kblocks - 1, es_hist[-1])
        for j in range(2):
            t, o = es_hist[-1][2 * j], es_hist[-1][2 * j + 1]
            nc.tensor.matmul(out=psD[j][:, :], lhsT=ones_bf[:, :],
                             rhs=t[:, o:o + qblk], start=False, stop=True)

        # ---- outputs: PSUM -> SBUF (DVE half A, ACT half B) -> DMA; the
        # small den copies go first so the oden DMAs launch early
        opool = ctx.enter_context(tc.tile_pool(name="outs", bufs=1))
        oDa = opool.tile([1, qblk], F32, name="oDa")
        oDb = opool.tile([1, qblk], F32, name="oDb")
        nc.vector.tensor_copy(oDa[:, :], psD[0][:, :])
        nc.scalar.activation(oDb[:, :], psD[1][:, :],
                             mybir.ActivationFunctionType.Copy)
        nc.gpsimd.dma_start(out=odenA, in_=oDa[:, :])
        nc.gpsimd.dma_start(out=odenB, in_=oDb[:, :])
        oNa = opool.tile([128, qblk], F32, name="oNa")
        oNb = opool.tile([128, qblk], F32, name="oNb")
        nc.vector.tensor_copy(oNa[:, :], accA[:, :])
        nc.scalar.activation(oNb[:, :], accB[:, :],
                             mybir.ActivationFunctionType.Copy)
        nc.sync.dma_start(out=onumA, in_=oNa[:, :])
        nc.sync.dma_start(out=onumB, in_=oNb[:, :])

    nc.compile()
    return nc


def make_inputs(mag, phase, n_cores=8):
    """Host-side sharding/layout prep -> per-core (key-rotated) input maps."""
    import ml_dtypes
    bf16 = ml_dtypes.bfloat16
    n, d = mag.shape
    q = n // n_cores
    qblk = q // 2
    kblocks = n // 128
    mag = np.ascontiguousarray(mag, dtype=np.float32)
    phase = np.ascontiguousarray(phase, dtype=np.float32)

    a = mag * np.cos(phase)
    b = mag * np.sin(phase)
    abt_g = (np.concatenate([a.T, b.T], axis=0) * SQRT_A).astype(bf16)
    v_nat = np.concatenate([mag, phase], axis=1).astype(bf16)   # [n, 128]

    rest = n - q
    rest_w = [1024, 1024, 2048, rest - 4096] if rest > 4096 else [rest]
    vch_w = [4, 4, 8, 16, 16, 16]

    def tile_blocks(x):  # [n, m] -> [kblocks, 128, m] block-major
        m = x.shape[1]
        return x.reshape(kblocks, 128, m)

    in_maps = []
    for c in range(n_cores):
        r = c * q
        abt_c = np.roll(abt_g, -r, axis=1)
        vtb = tile_blocks(np.roll(v_nat, -r, axis=0))  # [kblocks, 128, 128]
        m = {"onesv": np.ones((128, 1), np.float32),
             "onesb": np.ones((128, 1), bf16)}
        vb = 0
        for i, w in enumerate(vch_w):
            ch = vtb[vb:vb + w]                        # [w, 128, 128]
            m[f"vt{i}"] = np.ascontiguousarray(
                ch.transpose(1, 0, 2).reshape(128, w * 128))
            vb += w
        for i in range(2):
            m[f"abq{i}"] = np.ascontiguousarray(
                abt_c[:, i * qblk:(i + 1) * qblk])
        off = q
        for i, w in enumerate(rest_w):
            m[f"abt{i}"] = np.ascontiguousarray(abt_c[:, off:off + w])
            off += w
        in_maps.append(m)
    return in_maps


def gather_outputs(results, n, d, n_cores=8):
    """Per-core transposed unnormalized sums + denominators -> full outputs."""
    new_mag = np.empty((n, d), np.float32)
    new_phase = np.empty((n, d), np.float32)
    q = n // n_cores
    qblk = q // 2
    for c in range(n_cores):
        r = results[c]
        for j, (nm, dn) in enumerate((("onumA", "odenA"), ("onumB", "odenB"))):
            onum = np.asarray(r[nm]).astype(np.float32)   # [128, qblk]
            den = np.asarray(r[dn]).astype(np.float32)    # [1, qblk]
            qsl = slice(c * q + j * qblk, c * q + (j + 1) * qblk)
            new_mag[qsl] = (onum[:64, :] / den).T
            new_phase[qsl] = (onum[64:128, :] / den).T
    return new_mag, new_phase


_PROGRAM_CACHE = {}


def _get_program(n, d, n_cores):
    key = (n, d, n_cores)
    if key not in _PROGRAM_CACHE:
        _PROGRAM_CACHE[key] = build_program(n=n, d=d, n_cores=n_cores)
    return _PROGRAM_CACHE[key]


def kernel(mag, phase):
    mag = np.asarray(mag, dtype=np.float32)
    phase = np.asarray(phase, dtype=np.float32)
    n, d = mag.shape
    n_cores = 8
    nc = _get_program(n, d, n_cores)
    in_maps = make_inputs(mag, phase, n_cores=n_cores)
    res = run_bass_kernel_spmd(nc, in_maps, list(range(n_cores)))
    return gather_outputs(res.results, n, d, n_cores=n_cores)
